# revision 10
# baseline (speedup 1.0000x reference)
"""Trainium2 Bass kernel for nn_AssembledBlock_6167573037591.

Mixture-of-expert CNN block: control net (GAP -> 1x1 -> relu -> 1x1 ->
softmax over 4 experts) produces per-(sample, out-channel) expert mixing
coefficients; three chained 3x3 convs (64->64 ch, 128x128 img, pad 1) run
with the per-sample mixed weights.

Distribution: pure data parallel over batch. B=16 samples over 8 cores ->
2 samples per core, full weights everywhere, no collectives.

Per-core plan:
  - The 2 samples are stacked on the 128 SBUF partitions (sample*64+ch).
  - Images live padded to 130x130 in SBUF so each 3x3 tap is a free-dim
    offset; conv = 9 accumulating matmuls per 4-row (512 px) PSUM tile with
    a block-diagonal [128,128] stationary weight (per-sample mixed), in
    float32r (1 cycle/row on the PE vs 4 for fp32).
  - Control module + expert weight mixing run on-chip (DVE tensor_scalar
    with per-partition coefficients), weights transposed to [c, oc] layout
    via PE-mode transposes.
"""

import os
import numpy as np

_STATE = {}

N_CORES = 8
S = 2            # samples per core
C = 64           # channels (in == out)
H = W = 128
HP = WP = 130    # padded
NP = HP * WP
E = 4            # experts
HID = 16
TEMP = 30.0
RPG = 4          # output rows per PSUM tile (4*128 = 512)
RG = H // RPG    # 32 row groups
NCHUNK = 8      # input DMA / pooling chunks per sample (16 rows each)


def _emit(tc, nc, dram, mybir, bass):
    from contextlib import ExitStack

    F32 = mybir.dt.float32
    BF16 = mybir.dt.bfloat16
    AF = mybir.ActivationFunctionType
    ALU = mybir.AluOpType

    xd, cw1d, cw2d, wd, bd, od, identd = dram

    ctx = ExitStack()
    with ctx:
        big = ctx.enter_context(tc.tile_pool(name="big", bufs=1))
        wpool = ctx.enter_context(tc.tile_pool(name="wts", bufs=1))
        rwpool = ctx.enter_context(tc.tile_pool(name="raww", bufs=8))
        small = ctx.enter_context(tc.tile_pool(name="small", bufs=1))
        mixp = ctx.enter_context(tc.tile_pool(name="mix", bufs=4))
        stgx = ctx.enter_context(tc.tile_pool(name="stgx", bufs=8))
        ostgp = ctx.enter_context(tc.tile_pool(name="ostg", bufs=4))
        cpsum = ctx.enter_context(tc.tile_pool(name="cpsum", bufs=6, space="PSUM"))
        tpsum = ctx.enter_context(tc.tile_pool(name="tpsum", bufs=2, space="PSUM"))

        # ---- persistent big buffers ----
        xpad = big.tile([128, NP], BF16)  # conv1 input / conv2 output
        ypad = big.tile([128, NP], BF16)  # conv1 output / conv2 input
        xv = xpad[:].rearrange("p (h w) -> p h w", h=HP, w=WP)
        yv = ypad[:].rearrange("p (h w) -> p h w", h=HP, w=WP)

        # mixed+transposed conv weights, block-diag per tap:
        # lw[l][:, 128*t : 128*(t+1)] = [[w_s0T, 0], [0, w_s1T]]
        lw = [wpool.tile([128, 9 * 128], BF16, name=f"lw{i}") for i in range(3)]

        ident = small.tile([128, 128], F32)
        cw1sb = small.tile([16, C], F32)
        cw2sb = [small.tile([128, HID], F32, name=f"cw2sb{i}") for i in range(2)]
        cw1T = small.tile([C, HID], F32)
        cw2T = small.tile([HID, E * C], F32)
        braw = [small.tile([128, E], F32, name=f"braw{i}") for i in range(3)]
        bmix = [small.tile([128, 1], F32, name=f"bmix{i}") for i in range(3)]
        psums = small.tile([128, NCHUNK], F32)   # pooling partials
        pooled = small.tile([128, 1], F32)
        pooleds = small.tile([128, 1], F32)      # scaled by 1/(H*W)
        pooled2 = small.tile([C, S], F32)        # [c, s] for ctrl matmul
        hid_sb = small.tile([HID, S], F32)
        exp_sb = small.tile([C, S * E], F32)     # col = 4*s + e
        ssum = small.tile([C, S], F32)
        srec = small.tile([C, S], F32)
        coeff = small.tile([C, S * E], F32)      # col = 4*s + e
        coeff2 = small.tile([128, E], F32)       # partition = 64*s + oc


        # zero halo borders of both big buffers
        for v in (xv, yv):
            nc.vector.memset(v[:, 0, :], 0.0)
            nc.vector.memset(v[:, HP - 1, :], 0.0)
            nc.vector.memset(v[:, 1:HP - 1, 0:1], 0.0)
            nc.vector.memset(v[:, 1:HP - 1, WP - 1:WP], 0.0)
        for l in range(3):
            nc.vector.memset(lw[l][:], 0.0)

        # ---- input DMA + cast to bf16 + pooling (chunked, pipelined) ----
        rows_per_chunk = H // NCHUNK
        for rb in range(NCHUNK):
            r0 = rb * rows_per_chunk
            xstg = stgx.tile([128, rows_per_chunk * W], F32, name="xstg", tag="xstg")
            xsv = xstg[:].rearrange("p (h w) -> p h w", h=rows_per_chunk, w=W)
            for s in range(S):
                eng = nc.sync if (2 * rb + s) % 2 == 0 else nc.scalar
                eng.dma_start(
                    out=xsv[64 * s:64 * s + 64, :, :],
                    in_=xd.ap()[s, :, r0:r0 + rows_per_chunk, :],
                )
            # cast f32 -> bf16 into the padded buffer; accum_out = row sums
            nc.scalar.activation(
                xv[:, 1 + r0:1 + r0 + rows_per_chunk, 1:1 + W],
                xsv[:, :, :],
                AF.Copy,
                accum_out=psums[:, rb:rb + 1],
            )
        # ---- constants / static loads (issued on gpsimd queue) ----
        nc.gpsimd.dma_start(out=ident[:], in_=identd.ap())
        nc.gpsimd.dma_start(out=cw1sb[:], in_=cw1d.ap())
        nc.gpsimd.dma_start(out=cw2sb[0][:], in_=cw2d.ap()[0:128, :])
        nc.gpsimd.dma_start(out=cw2sb[1][:], in_=cw2d.ap()[128:256, :])
        for l in range(3):
            bT = bd[l].ap().rearrange("e o -> o e")   # tiny AP-swap dma
            nc.gpsimd.dma_start(out=braw[l][0:64, :], in_=bT)
            nc.gpsimd.dma_start(out=braw[l][64:128, :], in_=bT)

        def load_rw(l):
            rwl = []
            for e in range(E):
                t = rwpool.tile([C, C * 9], F32, name=f"rw{l}_{e}", tag="rw")
                wsrc = wd[l].ap()[e].rearrange("o c kh kw -> o (c kh kw)")
                nc.gpsimd.dma_start(out=t[:], in_=wsrc)
                tb = rwpool.tile([C, C * 9], BF16, name=f"rwb{l}_{e}", tag="rwb")
                nc.gpsimd.tensor_copy(tb[:], t[:])
                rwl.append(tb)
            return rwl

        rw = {0: load_rw(0)}

        nc.vector.tensor_reduce(
            out=pooled[:], in_=psums[:], axis=mybir.AxisListType.X, op=ALU.add
        )
        nc.scalar.mul(pooleds[:], pooled[:], 1.0 / (H * W))
        nc.vector.tensor_copy(pooled2[:, 0:1], pooleds[0:64, :])
        nc.sync.dma_start(out=pooled2[:, 1:2], in_=pooleds[64:128, :])

        # ---- transpose control weights on PE ----
        tp = tpsum.tile([C, HID], F32, tag="tp")
        nc.tensor.transpose(tp[:], cw1sb[:], ident[0:HID, 0:HID])
        nc.vector.tensor_copy(cw1T[:], tp[:])
        for h in range(2):
            tp2 = tpsum.tile([HID, 128], F32, tag="tp")
            nc.tensor.transpose(tp2[:], cw2sb[h][:], ident[:, :])
            nc.vector.tensor_copy(cw2T[:, 128 * h:128 * (h + 1)], tp2[:])

        # ---- control module ----
        hid_ps = cpsum.tile([HID, S], F32, tag="conv")
        nc.tensor.matmul(hid_ps[:], cw1T[:], pooled2[:], start=True, stop=True)
        nc.scalar.activation(hid_sb[:], hid_ps[:], AF.Relu)
        cw2T3 = cw2T[:].rearrange("p (o e) -> p o e", e=E)
        for e in range(E):
            lg = cpsum.tile([C, S], F32, tag="conv")
            nc.tensor.matmul(lg[:], cw2T3[:, :, e], hid_sb[:], start=True, stop=True)
            # exp(logit/TEMP); col layout 4*s + e
            nc.scalar.activation(
                exp_sb[:].rearrange("p (s e) -> p e s", e=E)[:, e],
                lg[:], AF.Exp, scale=1.0 / TEMP,
            )
        for s in range(S):
            nc.vector.tensor_reduce(
                out=ssum[:, s:s + 1], in_=exp_sb[:, E * s:E * (s + 1)],
                axis=mybir.AxisListType.X, op=ALU.add,
            )
        nc.vector.reciprocal(srec[:], ssum[:])
        for s in range(S):
            nc.vector.tensor_scalar_mul(
                coeff[:, E * s:E * (s + 1)], exp_sb[:, E * s:E * (s + 1)],
                srec[:, s:s + 1],
            )
        nc.vector.tensor_copy(coeff2[0:64, :], coeff[:, 0:E])
        nc.sync.dma_start(out=coeff2[64:128, :], in_=coeff[:, E:2 * E])

        # ---- mix expert weights + biases, transpose to [c, oc] block-diag ----
        identb = small.tile([C, C], BF16)
        nc.vector.tensor_copy(identb[:], ident[0:C, 0:C])
        for l in range(3):
            if l not in rw:
                rw[l] = load_rw(l)
            # per-sample mixed weights, bf16, both on partitions 0..63
            aggs = []
            for s in range(S):
                agg_a = mixp.tile([C, C * 9], BF16, tag="agg")
                nc.vector.tensor_scalar_mul(
                    agg_a[:], rw[l][0][:], coeff[:, E * s:E * s + 1]
                )
                for e in range(1, E):
                    agg_b = mixp.tile([C, C * 9], BF16, tag="agg")
                    nc.vector.scalar_tensor_tensor(
                        out=agg_b[:], in0=rw[l][e][:],
                        scalar=coeff[:, E * s + e:E * s + e + 1],
                        in1=agg_a[:], op0=ALU.mult, op1=ALU.add,
                    )
                    agg_a = agg_b
                aggs.append(agg_a)
            bt = mixp.tile([128, E], F32, tag="bt")
            nc.vector.tensor_tensor(bt[:], braw[l][:], coeff2[:], op=ALU.mult)
            nc.vector.tensor_reduce(
                out=bmix[l][:], in_=bt[:], axis=mybir.AxisListType.X, op=ALU.add
            )
            agg0 = aggs[0][:].rearrange("p (c t) -> p c t", t=9)
            agg1 = aggs[1][:].rearrange("p (c t) -> p c t", t=9)
            for t in range(9):
                tpa = tpsum.tile([C, C], BF16, tag="tp")
                nc.tensor.matmul(
                    tpa[:], agg0[:, :, t], identb[:],
                    is_transpose=True, start=True, stop=True,
                )
                tpb = tpsum.tile([128, C], BF16, tag="tp")
                nc.tensor.matmul(
                    tpb[64:128, :], agg1[:, :, t], identb[:],
                    is_transpose=True, tile_position=(0, 64),
                    start=True, stop=True,
                )
                nc.vector.tensor_copy(
                    lw[l][0:64, 128 * t:128 * t + 64], tpa[:]
                )
                nc.vector.tensor_copy(
                    lw[l][64:128, 128 * t + 64:128 * t + 128], tpb[64:128, :]
                )

        # ---- the three convs ----
        odv = od.ap().rearrange("s c h w -> (s c) h w")
        for l in range(3):
            srcv = (xv, yv, xv)[l]
            dstv = (yv, xv, None)[l]
            for rg in range(RG):
                ps = cpsum.tile([128, RPG * W], F32, tag="conv")
                for t in range(9):
                    dy, dx = divmod(t, 3)
                    rhs = srcv[:, RPG * rg + dy:RPG * rg + dy + RPG, dx:dx + W]
                    nc.tensor.matmul(
                        ps[:],
                        lw[l][:, 128 * t:128 * (t + 1)],
                        rhs,
                        start=(t == 0), stop=(t == 8),
                    )
                if l < 2:
                    dst = dstv[:, RPG * rg + 1:RPG * rg + 1 + RPG, 1:1 + W]
                    psv = ps[:].rearrange("p (h w) -> p h w", h=RPG, w=W)
                    if l == 0:
                        nc.vector.tensor_scalar_add(dst, psv, bmix[l][:, 0:1])
                    else:
                        nc.scalar.activation(
                            dst, psv, AF.Identity, bias=bmix[l][:, 0:1]
                        )
                else:
                    ostg = ostgp.tile([128, RPG * W], F32)
                    nc.scalar.activation(
                        ostg[:], ps[:], AF.Identity, bias=bmix[l][:, 0:1]
                    )
                    nc.sync.dma_start(
                        out=odv[:, RPG * rg:RPG * rg + RPG, :],
                        in_=ostg[:].rearrange("p (h w) -> p h w", h=RPG, w=W),
                    )


def _get_nc():
    if "nc" in _STATE:
        return _STATE["nc"]
    import concourse.bass as bass
    import concourse.tile as tile
    from concourse import bacc, mybir

    F32 = mybir.dt.float32
    nc = bacc.Bacc(
        "TRN2", target_bir_lowering=False, debug=False, num_devices=N_CORES
    )
    xd = nc.dram_tensor("x", [S, C, H, W], F32, kind="ExternalInput")
    cw1d = nc.dram_tensor("cw1", [HID, C], F32, kind="ExternalInput")
    cw2d = nc.dram_tensor("cw2", [E * C, HID], F32, kind="ExternalInput")
    wd = [
        nc.dram_tensor(f"w{l+1}", [E, C, C, 3, 3], F32, kind="ExternalInput")
        for l in range(3)
    ]
    bd = [
        nc.dram_tensor(f"b{l+1}", [E, C], F32, kind="ExternalInput")
        for l in range(3)
    ]
    od = nc.dram_tensor("out", [S, C, H, W], F32, kind="ExternalOutput")
    identd = nc.inline_tensor(np.eye(128, dtype=np.float32), name="ident128")

    with tile.TileContext(nc) as tc:
        _emit(tc, nc, (xd, cw1d, cw2d, wd, bd, od, identd), mybir, bass)
    nc.compile()
    _STATE["nc"] = nc
    return nc


def kernel(**inputs):
    from concourse.bass_utils import run_bass_kernel_spmd

    nc = _get_nc()
    arr = {
        k: np.ascontiguousarray(np.asarray(v, dtype=np.float32))
        for k, v in inputs.items()
    }
    x = arr["x"]
    shared = {k: v for k, v in arr.items() if k != "x"}
    in_maps = [
        {"x": np.ascontiguousarray(x[S * i:S * (i + 1)]), **shared}
        for i in range(N_CORES)
    ]
    trace = bool(int(os.environ.get("KBENCH_TRACE", "0")))
    res = run_bass_kernel_spmd(nc, in_maps, list(range(N_CORES)), trace=trace)
    _STATE["last"] = res
    return np.concatenate(
        [res.results[i]["out"] for i in range(N_CORES)], axis=0
    )
